# revision 1
# baseline (speedup 1.0000x reference)
"""CapsRoute Trainium2 kernel: grouped conv -> capsule self-routing -> grouped conv."""
import numpy as np
import concourse.bass as bass
import concourse.bacc as bacc
import concourse.tile as tile
from concourse import mybir
from concourse._compat import with_exitstack
from contextlib import ExitStack

K_CAT, P_CAT, K_OUT, P_OUT = 16, 8, 16, 8
C_CAT = 144
EPS_ROUTE = 1e-6
BN_EPS = 1e-5
H = W = 96
HP = WP = 98
ROWS_PER_CHUNK = 5
CHUNKS = [(r, min(ROWS_PER_CHUNK, H - r)) for r in range(0, H, ROWS_PER_CHUNK)]
NCHUNK = len(CHUNKS)
N = ROWS_PER_CHUNK * W  # 480 (tiles sized for the largest chunk)
ACT_FLUSH = {60: 0}  # chunk-end row -> flush range start
ACT_SPLIT_ROW = 60
PSN = 512  # PSUM tiles padded to a full 2KB bank to keep zero-regions private

F32 = mybir.dt.float32
F32R = mybir.dt.float32r
BF16 = mybir.dt.bfloat16
AF = mybir.ActivationFunctionType
USE_NATIVE_SILU = True
ALU = mybir.AluOpType


def prep_constants(conv_route_w, conv_route_gamma, conv_route_beta,
                   W_pose, W_gate, b_gate, spagg_w, spagg_gamma, spagg_beta):
    """Host-side constant prep. All lhsT arrays laid out [K_partition, free...]."""
    out = {}
    scale1 = (conv_route_gamma / np.sqrt(1.0 + BN_EPS)).astype(np.float32)
    scale2 = (spagg_gamma / np.sqrt(1.0 + BN_EPS)).astype(np.float32)

    # conv1 lhsT [72, 2, 9, 72]: [cin_local, half, tap, col j]
    # col j of conv1 psum_h: j<64 -> (k_loc=j//8, ch=j%8); j>=64 -> (k_loc=j-64, ch=8)
    c1 = np.zeros((72, 2, 9, 72), np.float32)
    w1 = conv_route_w * scale1[:, None, None, None]  # [144, 9, 3, 3]
    b1 = np.zeros((2, 72), np.float32)
    for h in range(2):
        for j in range(72):
            k_loc, ch = (j // 8, j % 8) if j < 64 else (j - 64, 8)
            cout = 72 * h + 9 * k_loc + ch
            for ci in range(9):
                for dy in range(3):
                    for dx in range(3):
                        c1[9 * k_loc + ci, h, 3 * dy + dx, j] = w1[cout, ci, dy, dx]
            b1[h, j] = conv_route_beta[cout]
    out["c1w"] = c1
    out["b1"] = b1

    # logits lhsT [64, 2, 128]: row 8*k_loc+p, [half], col 16*k_loc+o
    wg = np.zeros((64, 2, 128), np.float32)
    for h in range(2):
        for k_loc in range(8):
            for p in range(8):
                for o in range(16):
                    wg[8 * k_loc + p, h, 16 * k_loc + o] = W_gate[8 * h + k_loc, o, p]
    out["wg"] = wg
    out["bg"] = np.ascontiguousarray(b_gate.reshape(K_CAT, K_OUT)).astype(np.float32)

    ones_S = np.zeros((128, 8), np.float32)
    rep_t = np.zeros((72, 128), np.float32)
    ones_as = np.zeros((128, 16), np.float32)
    rep_r = np.zeros((16, 128), np.float32)
    sel = np.zeros((128, 16, 64), np.float32)  # [row, o, col]
    for k_loc in range(8):
        for o in range(16):
            ones_S[16 * k_loc + o, k_loc] = 1.0
            rep_t[64 + k_loc, 16 * k_loc + o] = 1.0
            ones_as[16 * k_loc + o, o] = 1.0
            rep_r[o, 16 * k_loc + o] = 1.0
            for p in range(8):
                sel[16 * k_loc + o, o, 8 * k_loc + p] = 1.0
    out["ones_S"] = ones_S
    out["rep_t"] = rep_t
    out["ones_as"] = ones_as
    out["rep_r"] = rep_r
    # per-quad 1/arsum replication selector for the normalized po4->SBUF stage
    rsel = np.zeros((16, 4, 128), np.float32)
    for quad in range(4):
        for j in range(4):
            for q in range(8):
                rsel[4 * quad + j, quad, 32 * j + q] = 1.0
    out["rsel"] = rsel
    out["sel"] = sel

    # wpose lhsT [128, 16, 8]: row 8k+p, [o], col q
    wp = np.zeros((128, 16, 8), np.float32)
    for o in range(16):
        for k in range(16):
            for p in range(8):
                wp[8 * k + p, o, :] = W_pose[k, o, p, :]
    out["wpose"] = wp

    # conv2 lhsT [72, 2, 9, 72] natural channel order
    c2 = np.zeros((72, 2, 9, 72), np.float32)
    w2 = spagg_w * scale2[:, None, None, None]
    for h in range(2):
        for j in range(72):
            cout = 72 * h + j
            g_loc = j // 9
            for ci in range(9):
                for dy in range(3):
                    for dx in range(3):
                        c2[9 * g_loc + ci, h, 3 * dy + dx, j] = w2[cout, ci, dy, dx]
    out["c2w"] = c2
    out["b2"] = spagg_beta.reshape(2, 72).astype(np.float32)
    for name, (shape, dt) in CONST_SPECS.items():
        want = mybir.dt.np(dt)
        out[name] = np.ascontiguousarray(out[name]).astype(want)
    return out


CONST_SPECS = {
    "c1w": ([72, 2, 9, 72], BF16),
    "b1": ([2, 72], F32),
    "wg": ([64, 2, 128], BF16),
    "bg": ([16, 16], F32),
    "ones_S": ([128, 8], BF16),
    "rep_t": ([72, 128], BF16),
    "ones_as": ([128, 16], BF16),
    "rep_r": ([16, 128], BF16),
    "rsel": ([16, 4, 128], BF16),
    "sel": ([128, 16, 64], BF16),
    "wpose": ([128, 16, 8], BF16),
    "c2w": ([72, 2, 9, 72], BF16),
    "b2": ([2, 72], F32),
}
BF16_NP = mybir.dt.np(BF16)


@with_exitstack
def capsroute_kernel(ctx: ExitStack, tc: tile.TileContext, outs, ins):
    nc = tc.nc
    out = outs["out"]

    singles = ctx.enter_context(tc.tile_pool(name="singles", bufs=1))
    xpool = ctx.enter_context(tc.tile_pool(name="xpool", bufs=1))
    y2pool = ctx.enter_context(tc.tile_pool(name="y2pool", bufs=1))
    work = ctx.enter_context(tc.tile_pool(name="work", bufs=3))
    rwork = ctx.enter_context(tc.tile_pool(name="rwork", bufs=3))
    psum = ctx.enter_context(tc.tile_pool(name="psum", bufs=2, space="PSUM"))

    cst = {}
    for name, (shape, dt) in CONST_SPECS.items():
        if name in ("b1", "b2", "bg"):
            continue  # loaded as column vectors below
        t = singles.tile(shape, dt, name=f"{name}_c")
        nc.sync.dma_start(out=t[:], in_=ins[name][:])
        cst[name] = t

    wg_b, ones_S_b, rep_t_b, ones_as_b, rep_r_b, sel_b, wpose_b = (
        cst["wg"], cst["ones_S"], cst["rep_t"], cst["ones_as"], cst["rep_r"],
        cst["sel"], cst["wpose"])

    # b_gate per half as [128,1] column vectors (row 16*k_loc+o)
    bg_t = []
    for h in range(2):
        t = singles.tile([128, 1], F32, name=f"bg{h}")
        nc.sync.dma_start(
            out=t[:], in_=ins["bg"][8 * h:8 * h + 8, :].rearrange("a b -> (a b)").unsqueeze(1))
        bg_t.append(t)
    b1_t = [singles.tile([72, 1], F32, name=f"b1_{h}") for h in range(2)]
    b2_t = [singles.tile([72, 1], F32, name=f"b2_{h}") for h in range(2)]
    for h in range(2):
        nc.sync.dma_start(out=b1_t[h][:], in_=ins["b1"][h:h + 1, :].transpose([1, 0]))
        nc.sync.dma_start(out=b2_t[h][:], in_=ins["b2"][h:h + 1, :].transpose([1, 0]))

    def silu(out_ap, psum_ap, bias_ap):
        if USE_NATIVE_SILU:
            nc.scalar.activation(out_ap, psum_ap, AF.Silu, bias=bias_ap)
        else:
            # CoreSim-compatible decomposition: sigmoid then (x+b)*sig fused.
            tmp = rwork.tile([psum_ap.tensor.shape[0], N], F32, tag="silu_tmp",
                             name=f"silu_tmp{nc.next_id()}")
            fs = psum_ap.free_size()
            bp = psum_ap.base_partition()
            t_ap = tmp[bp:bp + out_ap.shape[0], 0:fs]
            nc.scalar.activation(t_ap, psum_ap, AF.Sigmoid, bias=bias_ap)
            with nc.allow_low_precision(reason="silu bf16 out"):
                nc.vector.scalar_tensor_tensor(out_ap, psum_ap, bias_ap, t_ap,
                                               op0=ALU.add, op1=ALU.mult)

    def win(t, r0, nr, dy, dx):
        rs = 1 + r0 + dy
        return t[:, rs:rs + nr, 1 + dx:1 + dx + W]

    def pad_border(t):
        nc.vector.memset(t[:, 0, :], 0.0)
        nc.vector.memset(t[:, 97, :], 0.0)
        nc.vector.memset(t[:, :, 0:1], 0.0)
        nc.vector.memset(t[:, :, 97:98], 0.0)

    zmm = singles.tile([1, 128], BF16, name="zmm")
    nc.vector.memset(zmm[:], 0.0)
    zrhs = singles.tile([1, N], BF16, name="zrhs")
    nc.vector.memset(zrhs[:], 0.0)
    onerhs = singles.tile([1, N], BF16, name="onerhs")
    nc.vector.memset(onerhs[:], 1.0)
    epsw = singles.tile([1, 16], BF16, name="epsw")
    nc.vector.memset(epsw[:], EPS_ROUTE)

    def zero_psum(ps, nrows, NC):
        nc.tensor.matmul(ps[0:nrows, 0:NC], zmm[0:1, 0:nrows], zrhs[:, 0:NC],
                         start=True, stop=False, skip_group_check=True)

    eps_t = singles.tile([16, 1], F32, name="eps_t")
    nc.vector.memset(eps_t[:], EPS_ROUTE)
    xpad = [xpool.tile([72, HP, WP], BF16, name=f"xpad{h}") for h in range(2)]
    for h, xsrc in enumerate((ins["x0"], ins["x1"])):
        pad_border(xpad[h])
        nc.gpsimd.dma_start(out=xpad[h][:, 1:97, 1:97], in_=xsrc[:])

    y2 = [y2pool.tile([72, HP, WP], BF16, name=f"y2{h}") for h in range(2)]
    for h in range(2):
        pad_border(y2[h])
    as_img = y2pool.tile([16, H * W], BF16, name="as_img")

    def conv2_chunk(c):
        r0, nr = CHUNKS[c]
        NC = nr * W
        for h in range(2):
            ps = psum.tile([72, PSN], F32, tag="c2ps", name="c2ps", bufs=2)
            for tap in range(9):
                dy, dx = tap // 3 - 1, tap % 3 - 1
                nc.tensor.matmul(
                    ps[:, 0:NC], cst["c2w"][:, h, tap],
                    win(y2[h], r0, nr, dy, dx),
                    start=(tap == 0), stop=(tap == 8))
            ob = work.tile([72, N], F32, tag="ob")
            silu(ob[:, 0:NC], ps[:, 0:NC], b2_t[h][:])
            nc.sync.dma_start(
                out=out[72 * h:72 * h + 72, r0:r0 + nr, :],
                in_=ob[:, 0:NC].rearrange("p (r w) -> p r w", w=W))

    # ============ conv1 + routing, conv2 interleaved (lag 1) ============
    for c in range(NCHUNK):
        r0, nr = CHUNKS[c]
        NC = nr * W
        ps_h = []
        for h in range(2):
            ps = psum.tile([72, PSN], F32, tag="c1ps", bufs=2)
            for tap in range(9):
                dy, dx = tap // 3 - 1, tap % 3 - 1
                nc.tensor.matmul(
                    ps[:, 0:NC], cst["c1w"][:, h, tap],
                    win(xpad[h], r0, nr, dy, dx),
                    start=(tap == 0), stop=(tap == 8))
            ps_h.append(ps)
        pose = rwork.tile([128, N], BF16, tag="pose", bufs=4)
        # acty chain lives at base-64 slices ([72,N] tiles) so every op is
        # same-start-partition; walrus rejects cross-base element ops.
        acty = [rwork.tile([72, N], F32, name=f"acty{h}", tag=f"acty{h}") for h in range(2)]
        for h in range(2):
            if h == 0:
                silu(pose[0:64, 0:NC], ps_h[0][0:64, 0:NC], b1_t[0][0:64, :])
                pose_src = [pose]
            else:
                ptmp = rwork.tile([64, N], BF16, tag="ptmp")
                silu(ptmp[:, 0:NC], ps_h[1][0:64, 0:NC], b1_t[1][0:64, :])
                # only the cp-mults need the combined [128,N] pose tile; logits
                # reads ptmp directly so this DMA overlaps the routing head.
                nc.sync.dma_start(out=pose[64:128, 0:NC], in_=ptmp[:, 0:NC])
                pose_src.append(ptmp)
            silu(acty[h][64:72, 0:NC], ps_h[h][64:72, 0:NC], b1_t[h][64:72, :])
        # routing
        E = []
        Sts = []
        for h in range(2):
            L = psum.tile([128, PSN], F32, tag="big", name="L", bufs=2)
            nc.tensor.matmul(L[:, 0:NC], wg_b[:, h],
                             pose_src[h][0:64, 0:NC], start=True, stop=True)
            Eh = rwork.tile([128, N], BF16, tag=f"E{h}", bufs=4)
            nc.scalar.activation(Eh[:, 0:NC], L[:, 0:NC], AF.Exp, bias=bg_t[h][:])
            E.append(Eh)
            # S at rows 64:72 so the whole act chain (sigmoid/recip/mul) shares
            # the conv-psum act-row base and needs no partition-moving DMA.
            Sth = psum.tile([72, PSN], F32, tag="psmall", name=f"St{h}", bufs=1)
            nc.tensor.matmul(Sth[64:72, 0:NC], ones_S_b[:], Eh[:, 0:NC],
                             start=True, stop=True)
            Sts.append(Sth)
        ar = []
        for h in range(2):
            sl = slice(64, 72)
            sg = rwork.tile([72, N], F32, name=f"sg{h}", tag=f"sg{h}")
            nc.scalar.activation(sg[sl, 0:NC], acty[h][sl, 0:NC], AF.Sigmoid)
            rS = rwork.tile([72, N], F32, name=f"rS{h}", tag=f"rS{h}")
            nc.vector.reciprocal(rS[sl, 0:NC], Sts[h][sl, 0:NC])
            th = rwork.tile([72, N], BF16, name=f"t{h}", tag=f"t{h}")
            with nc.allow_low_precision(reason="bf16 routing coefficients"):
                nc.vector.tensor_mul(th[sl, 0:NC], sg[sl, 0:NC], rS[sl, 0:NC])
            rtp = psum.tile([128, PSN], F32, tag="psmall", name="rtp", bufs=1)
            nc.tensor.matmul(rtp[:, 0:NC], rep_t_b[sl, :], th[sl, 0:NC],
                             start=True, stop=True)
            arh = rwork.tile([128, N], BF16, name=f"ar{h}", tag=f"ar{h}", bufs=4)
            with nc.allow_low_precision(reason="bf16 routing coefficients"):
                nc.vector.tensor_mul(arh[:, 0:NC], E[h][:, 0:NC], rtp[:, 0:NC])
            ar.append(arh)
        asum = psum.tile([128, PSN], F32, tag="psmall", name="asum", bufs=1)
        for h in range(2):
            nc.tensor.matmul(asum[0:16, 0:NC], ones_as_b[:], ar[h][:, 0:NC],
                             start=(h == 0), stop=(h == 1))
        as_eps = rwork.tile([16, N], F32, tag="as_eps")
        nc.vector.tensor_scalar_add(as_eps[:, 0:NC], asum[0:16, 0:NC], EPS_ROUTE)
        r = rwork.tile([16, N], BF16, tag="r")
        with nc.allow_low_precision(reason="bf16 routing coefficients"):
            nc.vector.reciprocal(r[:, 0:NC], as_eps[:, 0:NC])
            nc.vector.tensor_copy(as_img[:, r0 * W:r0 * W + NC], as_eps[:, 0:NC])
        if (r0 + nr) in ACT_FLUSH:
            # progressive flush of finished act rows so conv2 chunks can start
            # while routing still runs (y2 act rows are the only late deps).
            lo = ACT_FLUSH[r0 + nr]
            hi = r0 + nr
            for o in range(16):
                h2, o_loc = o // 8, o % 8
                eng = nc.scalar if o % 2 == 0 else nc.sync
                eng.dma_start(
                    out=y2[h2][9 * o_loc + 8:9 * o_loc + 9, 1 + lo:1 + hi, 1:97],
                    in_=as_img[o:o + 1, lo * W:hi * W].rearrange("p (r w) -> p r w", w=W))
        # o-loop votes on pre-division ar; 1/arsum applied at the po4->SBUF
        # stage below, so the 16-capsule chain never waits on the reciprocal.
        for quad in range(4):
            po4 = psum.tile([128, PSN], F32, tag="po4", bufs=1)
            zero_psum(po4, 128, NC)
            for j in range(4):
                o = 4 * quad + j
                rep = psum.tile([128, PSN], F32, tag="big", name="rep", bufs=2)
                for h in range(2):
                    nc.tensor.matmul(rep[64 * h:64 * h + 64, 0:NC], sel_b[:, o],
                                     ar[h][:, 0:NC], start=True, stop=True)
                cp = rwork.tile([128, N], BF16, tag="cp")
                nc.vector.tensor_mul(cp[:, 0:NC], pose[:, 0:NC], rep[:, 0:NC])
                nc.tensor.matmul(po4[32 * j:32 * j + 8, 0:NC], wpose_b[:, o],
                                 cp[:, 0:NC], start=False, stop=True,
                                 skip_group_check=True, tile_position=(0, 32 * j))
            rrep = psum.tile([128, PSN], F32, tag="psmall", name="rrep", bufs=1)
            nc.tensor.matmul(rrep[:, 0:NC], cst["rsel"][:, quad], r[:, 0:NC],
                             start=True, stop=True)
            rrep_sb = rwork.tile([128, N], BF16, tag="rrepsb")
            with nc.allow_low_precision(reason="bf16 conv2 input"):
                nc.scalar.copy(rrep_sb[:, 0:NC], rrep[:, 0:NC])
            po4_sb = rwork.tile([128, N], BF16, tag="po4sb")
            with nc.allow_low_precision(reason="bf16 conv2 input"):
                nc.vector.tensor_mul(po4_sb[:, 0:NC], rrep_sb[:, 0:NC], po4[:, 0:NC])
            for j in range(4):
                o = 4 * quad + j
                h2, o_loc = o // 8, o % 8
                eng = nc.scalar if j % 2 == 0 else nc.sync
                eng.dma_start(
                    out=win(y2[h2], r0, nr, 0, 0)[9 * o_loc:9 * o_loc + 8],
                    in_=po4_sb[32 * j:32 * j + 8, 0:NC].rearrange("p (r w) -> p r w", w=W))

    for o in range(16):
        h2, o_loc = o // 8, o % 8
        eng = nc.scalar if o % 2 == 0 else nc.sync
        eng.dma_start(
            out=y2[h2][9 * o_loc + 8:9 * o_loc + 9, 1 + ACT_SPLIT_ROW:97, 1:97],
            in_=as_img[o:o + 1, ACT_SPLIT_ROW * W:].rearrange("p (r w) -> p r w", w=W))
    for c in range(NCHUNK):
        conv2_chunk(c)


def build_nc():
    nc = bacc.Bacc("TRN2", target_bir_lowering=False, debug=False)
    ins = {
        "x0": nc.dram_tensor("x0", [72, H, W], BF16, kind="ExternalInput").ap(),
        "x1": nc.dram_tensor("x1", [72, H, W], BF16, kind="ExternalInput").ap(),
    }
    for name, (shape, dt) in CONST_SPECS.items():
        ins[name] = nc.dram_tensor(name, shape, dt, kind="ExternalInput").ap()
    outs = {"out": nc.dram_tensor("out", [C_CAT, H, W], F32, kind="ExternalOutput").ap()}
    with tile.TileContext(nc) as tc:
        capsroute_kernel(tc, outs, ins)
    nc.compile()
    return nc

# ======================= host-side runner =======================
_NC_CACHE = {}


def _get_nc():
    if "nc" not in _NC_CACHE:
        _NC_CACHE["nc"] = build_nc()
    return _NC_CACHE["nc"]


def kernel(**inputs):
    """Full-batch entry point: shards batch 8 across 8 NeuronCores."""
    from concourse import bass_utils

    nc = _get_nc()
    consts = prep_constants(
        inputs["conv_route_w"].astype(np.float32),
        inputs["conv_route_gamma"].astype(np.float32),
        inputs["conv_route_beta"].astype(np.float32),
        inputs["W_pose"].astype(np.float32),
        inputs["W_gate"].astype(np.float32),
        inputs["b_gate"].astype(np.float32),
        inputs["spagg_w"].astype(np.float32),
        inputs["spagg_gamma"].astype(np.float32),
        inputs["spagg_beta"].astype(np.float32),
    )
    x0 = np.asarray(inputs["x0"]).astype(BF16_NP)
    x1 = np.asarray(inputs["x1"]).astype(BF16_NP)
    in_maps = []
    for b in range(8):
        m = dict(consts)
        m["x0"] = np.ascontiguousarray(x0[b])
        m["x1"] = np.ascontiguousarray(x1[b])
        in_maps.append(m)
    res = bass_utils.run_bass_kernel_spmd(nc, in_maps, core_ids=list(range(8)))
    out = np.stack([res.results[b]["out"] for b in range(8)], axis=0)
    return out.astype(np.float32)



# revision 6
# speedup vs baseline: 1.5250x; 1.5250x over previous
"""CapsRoute Trainium2 kernel v2: o-major routing, direct-to-channel votes.

Structure per row-chunk (nr rows, NC = nr*96 pixels):
  conv1 (18 mm) -> silu -> [o-major logits (4 mm) -> exp -> S (4 mm) ->
  recip -> th -> rtp (4 mm) -> ar -> asum (2 mm) -> as_eps -> r ->
  bc (2 mm) -> coeff] -> per-o [rep mm -> cp mul -> vote mm into
  channel-ordered packed psum] + act-insert mm -> Act copy -> 2 scatter
  DMAs per half into y2 -> conv2 (18 mm) at lag 3.
"""
import numpy as np
import concourse.bass as bass
import concourse.bacc as bacc
import concourse.tile as tile
from concourse import mybir
from concourse._compat import with_exitstack
from contextlib import ExitStack

K_CAT, P_CAT, K_OUT, P_OUT = 16, 8, 16, 8
C_CAT = 144
EPS_ROUTE = 1e-6
BN_EPS = 1e-5
H = W = 96
HP = WP = 98
ROWS_PER_CHUNK = 5
CHUNKS = [(5 * i, 5) for i in range(16)] + [(80 + 4 * i, 4) for i in range(4)]
NCHUNK = len(CHUNKS)
N = ROWS_PER_CHUNK * W  # 480
PSN = 512
C2LAG = 3

F32 = mybir.dt.float32
BF16 = mybir.dt.bfloat16
AF = mybir.ActivationFunctionType
ALU = mybir.AluOpType


def prep_constants(conv_route_w, conv_route_gamma, conv_route_beta,
                   W_pose, W_gate, b_gate, spagg_w, spagg_gamma, spagg_beta):
    out = {}
    scale1 = (conv_route_gamma / np.sqrt(1.0 + BN_EPS)).astype(np.float32)
    scale2 = (spagg_gamma / np.sqrt(1.0 + BN_EPS)).astype(np.float32)

    # conv1 lhsT [72, 2, 9, 72]: [cin_local, half, tap, col j]
    # col j: j<64 -> pose (k_loc=j//8, p=j%8); j>=64 -> act of k_loc=j-64
    c1 = np.zeros((72, 2, 9, 72), np.float32)
    w1 = conv_route_w * scale1[:, None, None, None]
    b1 = np.zeros((2, 72), np.float32)
    for h in range(2):
        for j in range(72):
            k_loc, ch = (j // 8, j % 8) if j < 64 else (j - 64, 8)
            cout = 72 * h + 9 * k_loc + ch
            for ci in range(9):
                for dy in range(3):
                    for dx in range(3):
                        c1[9 * k_loc + ci, h, 3 * dy + dx, j] = w1[cout, ci, dy, dx]
            b1[h, j] = conv_route_beta[cout]
    out["c1w"] = c1
    out["b1"] = b1

    # o-major logits lhsT [64, 4, 128]: slots (half-tile, rhs-half):
    # 0=(A,pose0) 1=(A,ptmp) 2=(B,pose0) 3=(B,ptmp); row r=8*k_loc+p,
    # col j=16*o_loc+k
    wg4 = np.zeros((64, 4, 128), np.float32)
    for r in range(64):
        kl, p = r // 8, r % 8
        for o_loc in range(8):
            wg4[r, 0, 16 * o_loc + kl] = W_gate[kl, o_loc, p]
            wg4[r, 1, 16 * o_loc + 8 + kl] = W_gate[8 + kl, o_loc, p]
            wg4[r, 2, 16 * o_loc + kl] = W_gate[kl, 8 + o_loc, p]
            wg4[r, 3, 16 * o_loc + 8 + kl] = W_gate[8 + kl, 8 + o_loc, p]
    out["wg4"] = wg4

    bg = np.asarray(b_gate).reshape(K_CAT, K_OUT)
    bgA = np.zeros((128, 1), np.float32)
    bgB = np.zeros((128, 1), np.float32)
    for o_loc in range(8):
        for k in range(16):
            bgA[16 * o_loc + k, 0] = bg[k, o_loc]
            bgB[16 * o_loc + k, 0] = bg[k, 8 + o_loc]
    out["bgA"] = bgA
    out["bgB"] = bgB

    # S: sum E over o per k-half: rows 16o+k -> col k-8h
    onesS = np.zeros((128, 2, 8), np.float32)
    for o_loc in range(8):
        for k in range(16):
            h = k // 8
            onesS[16 * o_loc + k, h, k - 8 * h] = 1.0
    out["onesS"] = onesS

    # rtp broadcast: rows 64+kl (th rows) -> cols 16o+ (8h+kl); 0.5 folds
    # the tanh->sigmoid affine
    rt = np.zeros((72, 2, 128), np.float32)
    for h in range(2):
        for kl in range(8):
            for o_loc in range(8):
                rt[64 + kl, h, 16 * o_loc + 8 * h + kl] = 0.5
    out["rt"] = rt

    # asum: rows 16o+k -> col o
    onesA = np.zeros((128, 8), np.float32)
    for o_loc in range(8):
        for k in range(16):
            onesA[16 * o_loc + k, o_loc] = 1.0
    out["onesA"] = onesA

    # late-normalize broadcast: r rows (A 0:8 / B 32:40, row 8 = ones) ->
    # packed channel cols; act cols get the constant-1 row
    rsel2 = np.zeros((72, 2, 100), np.float32)
    for h in range(2):
        for o_loc in range(8):
            for q in range(8):
                col = 64 * (o_loc // 4) + 9 * (o_loc % 4) + q
                rsel2[32 * h + o_loc, h, col] = 1.0
        for g in range(2):
            for j in range(4):
                rsel2[64, h, 64 * g + 9 * j + 8] = 1.0
    out["rsel2"] = rsel2

    # rep broadcast: rows 16o'+k -> cols 8k+p (all p) for o'==o_loc
    sel2 = np.zeros((128, 8, 128), np.float32)
    for o_loc in range(8):
        for k in range(16):
            for p in range(8):
                sel2[16 * o_loc + k, o_loc, 8 * k + p] = 1.0
    out["sel2"] = sel2

    # votes straight into channel order: rows 8k+p, per o: col
    # 9*(o_loc%4)+q in group o_loc//4
    wp36 = np.zeros((128, 16, 36), np.float32)
    for o in range(16):
        o_loc = o % 8
        for k in range(16):
            for p in range(8):
                for q in range(8):
                    wp36[8 * k + p, o, 9 * (o_loc % 4) + q] = W_pose[k, o, p, q]
    out["wp36"] = wp36

    # act insertion: rows of as_eps (A 0:8 / B 32:40) -> cols {9j+8, 64+9j+8}
    actsel = np.zeros((40, 2, 100), np.float32)
    for h in range(2):
        for o_loc in range(8):
            col = 64 * (o_loc // 4) + 9 * (o_loc % 4) + 8
            actsel[32 * h + o_loc, h, col] = 1.0
    out["actsel"] = actsel

    # conv2 lhsT [72, 2, 9, 72] natural channel order
    c2 = np.zeros((72, 2, 9, 72), np.float32)
    w2 = spagg_w * scale2[:, None, None, None]
    for h in range(2):
        for j in range(72):
            cout = 72 * h + j
            g_loc = j // 9
            for ci in range(9):
                for dy in range(3):
                    for dx in range(3):
                        c2[9 * g_loc + ci, h, 3 * dy + dx, j] = w2[cout, ci, dy, dx]
    out["c2w"] = c2
    out["b2"] = spagg_beta.reshape(2, 72).astype(np.float32)
    for name, (shape, dt) in CONST_SPECS.items():
        want = mybir.dt.np(dt)
        out[name] = np.ascontiguousarray(out[name]).astype(want)
    return out


CONST_SPECS = {
    "c1w": ([72, 2, 9, 72], BF16),
    "b1": ([2, 72], F32),
    "wg4": ([64, 4, 128], BF16),
    "bgA": ([128, 1], F32),
    "bgB": ([128, 1], F32),
    "onesS": ([128, 2, 8], BF16),
    "rt": ([72, 2, 128], BF16),
    "onesA": ([128, 8], BF16),
    "rsel2": ([72, 2, 100], BF16),
    "sel2": ([128, 8, 128], BF16),
    "wp36": ([128, 16, 36], BF16),
    "actsel": ([40, 2, 100], BF16),
    "c2w": ([72, 2, 9, 72], BF16),
    "b2": ([2, 72], F32),
}
BF16_NP = mybir.dt.np(BF16)
XROWS = [(0, 6), (6, 5), (11, 24), (35, 24), (59, 24), (83, 13)]


@with_exitstack
def capsroute_kernel(ctx: ExitStack, tc: tile.TileContext, outs, ins):
    nc = tc.nc
    out = outs["out"]

    singles = ctx.enter_context(tc.tile_pool(name="singles", bufs=1))
    xpool = ctx.enter_context(tc.tile_pool(name="xpool", bufs=1))
    y2pool = ctx.enter_context(tc.tile_pool(name="y2pool", bufs=1))
    work = ctx.enter_context(tc.tile_pool(name="work", bufs=3))
    psc = ctx.enter_context(tc.tile_pool(name="psc", bufs=2, space="PSUM"))
    psl = ctx.enter_context(tc.tile_pool(name="psl", bufs=2, space="PSUM"))
    psr = ctx.enter_context(tc.tile_pool(name="psr", bufs=2, space="PSUM"))
    pck = ctx.enter_context(tc.tile_pool(name="pck", bufs=2, space="PSUM"))

    cst = {}
    for name, (shape, dt) in CONST_SPECS.items():
        if name in ("b1", "b2"):
            continue
        t = singles.tile(shape, dt, name=f"{name}_c")
        cst[name] = t
    b1_t = [singles.tile([72, 1], F32, name=f"b1_{h}") for h in range(2)]
    b2_t = [singles.tile([72, 1], F32, name=f"b2_{h}") for h in range(2)]

    r_t = singles.tile([72, N], BF16, name="r_t")
    nc.vector.memset(r_t[:], 0.0)
    nc.vector.memset(r_t[64:72, :], 1.0)
    xpad = [xpool.tile([72, HP, WP], BF16, name=f"xpad{h}") for h in range(2)]
    y2 = [y2pool.tile([72, HP, WP], BF16, name=f"y2{h}") for h in range(2)]

    def pad_border(t):
        nc.vector.memset(t[:, 0, :], 0.0)
        nc.vector.memset(t[:, 97, :], 0.0)
        nc.vector.memset(t[:, :, 0:1], 0.0)
        nc.vector.memset(t[:, :, 97:98], 0.0)

    for h in range(2):
        pad_border(xpad[h])
        pad_border(y2[h])

    # load order tuned for startup: first x rows, conv1 consts, routing
    # consts, remaining x rows, conv2 consts
    nc.sync.dma_start(out=cst["c1w"][:, 0, 0:3], in_=ins["c1w"][:, 0, 0:3])
    nc.sync.dma_start(out=cst["c1w"][:, 0, 3:9], in_=ins["c1w"][:, 0, 3:9])
    r0, nr = XROWS[0]
    nc.gpsimd.dma_start(out=xpad[0][:, 1 + r0:1 + r0 + nr, 1:97],
                        in_=ins["x0"][:, r0:r0 + nr, :])
    nc.sync.dma_start(out=cst["c1w"][:, 1], in_=ins["c1w"][:, 1])
    nc.gpsimd.dma_start(out=xpad[1][:, 1 + r0:1 + r0 + nr, 1:97],
                        in_=ins["x1"][:, r0:r0 + nr, :])
    for h in range(2):
        nc.sync.dma_start(out=b1_t[h][:], in_=ins["b1"][h:h + 1, :].transpose([1, 0]))
    for name in ("wg4", "bgA", "bgB", "onesS", "rt", "onesA", "rsel2"):
        nc.sync.dma_start(out=cst[name][:], in_=ins[name][:])
    for h, xsrc in enumerate((ins["x0"], ins["x1"])):
        for r0, nr in XROWS[1:]:
            nc.gpsimd.dma_start(out=xpad[h][:, 1 + r0:1 + r0 + nr, 1:97],
                                in_=xsrc[:, r0:r0 + nr, :])
    for name in ("sel2", "wp36", "actsel"):
        nc.sync.dma_start(out=cst[name][:], in_=ins[name][:])
    nc.sync.dma_start(out=cst["c2w"][:], in_=ins["c2w"][:])
    for h in range(2):
        nc.sync.dma_start(out=b2_t[h][:], in_=ins["b2"][h:h + 1, :].transpose([1, 0]))

    def win(t, r0, nr, dy, dx):
        rs = 1 + r0 + dy
        return t[:, rs:rs + nr, 1 + dx:1 + dx + W]

    def conv2_mm(c):
        r0, nr = CHUNKS[c]
        NC = nr * W
        pss = []
        for h in range(2):
            ps = psc.tile([72, PSN], F32, tag="c2ps", name="c2ps")
            for tap in range(9):
                dy, dx = tap // 3 - 1, tap % 3 - 1
                nc.tensor.matmul(
                    ps[:, 0:NC], cst["c2w"][:, h, tap],
                    win(y2[h], r0, nr, dy, dx),
                    start=(tap == 0), stop=(tap == 8))
            pss.append(ps)
        return pss

    def conv2_fin(c, pss):
        r0, nr = CHUNKS[c]
        NC = nr * W
        for h in range(2):
            ob = work.tile([72, N], F32, tag="ob")
            nc.scalar.activation(ob[:, 0:NC], pss[h][:, 0:NC], AF.Silu, bias=b2_t[h][:])
            nc.sync.dma_start(
                out=out[72 * h:72 * h + 72, r0:r0 + nr, :],
                in_=ob[:, 0:NC].rearrange("p (r w) -> p r w", w=W))

    def scatter_chunk(c, pk_sb):
        r0, nr = CHUNKS[c]
        NC = nr * W
        for h in range(2):
            pk = pk_sb[h]
            for g in range(2):
                src = pk[64 * g:64 * g + 36, 0:NC].rearrange("p (r w) -> p r w", w=W)
                nc.sync.dma_start(
                    out=y2[h][36 * g:36 * g + 36, 1 + r0:1 + r0 + nr, 1:97],
                    in_=src)

    def routing_chunk(c, fin_prev, mm_lagged):
        r0, nr = CHUNKS[c]
        NC = nr * W
        # --- conv1 ---
        ps_h = []
        for h in range(2):
            ps = psc.tile([72, PSN], F32, tag="c1ps", name="c1ps")
            for tap in range(9):
                dy, dx = tap // 3 - 1, tap % 3 - 1
                nc.tensor.matmul(
                    ps[:, 0:NC], cst["c1w"][:, h, tap],
                    win(xpad[h], r0, nr, dy, dx),
                    start=(tap == 0), stop=(tap == 8))
            ps_h.append(ps)
        pose = work.tile([128, N], BF16, tag="pose", bufs=4)
        ptmp = work.tile([64, N], BF16, tag="ptmp")
        acty = [work.tile([72, N], F32, name=f"acty{h}", tag=f"acty{h}") for h in range(2)]
        nc.scalar.activation(pose[0:64, 0:NC], ps_h[0][0:64, 0:NC], AF.Silu,
                             bias=b1_t[0][0:64, :])
        nc.scalar.activation(ptmp[:, 0:NC], ps_h[1][0:64, 0:NC], AF.Silu,
                             bias=b1_t[1][0:64, :])
        for h in range(2):
            nc.scalar.activation(acty[h][64:72, 0:NC], ps_h[h][64:72, 0:NC],
                                 AF.Silu, bias=b1_t[h][64:72, :])
        nc.sync.dma_start(out=pose[64:128, 0:NC], in_=ptmp[:, 0:NC])
        if fin_prev is not None:
            conv2_fin(*fin_prev)
        # tanh(x/2) = 2*sigmoid(x)-1; shares the exp act-table set
        tt = [work.tile([72, N], BF16, name=f"t{h}", tag=f"t{h}") for h in range(2)]
        for h in range(2):
            with nc.allow_low_precision(reason="bf16 routing"):
                nc.scalar.activation(tt[h][64:72, 0:NC], acty[h][64:72, 0:NC],
                                     AF.Tanh, scale=0.5)
        # --- o-major logits + exp ---
        E = []
        for s, half in enumerate("AB"):
            L = psl.tile([128, PSN], F32, tag="L", name=f"L{half}")
            nc.tensor.matmul(L[:, 0:NC], cst["wg4"][:, 2 * s], pose[0:64, 0:NC],
                             start=True, stop=False)
            nc.tensor.matmul(L[:, 0:NC], cst["wg4"][:, 2 * s + 1], ptmp[:, 0:NC],
                             start=False, stop=True)
            Eh = work.tile([128, N], BF16, tag=f"E{half}", bufs=4)
            with nc.allow_low_precision(reason="bf16 routing"):
                nc.scalar.activation(Eh[:, 0:NC], L[:, 0:NC], AF.Exp,
                                     bias=cst["bg" + half][:])
            E.append(Eh)
        c2ps_cur = conv2_mm(mm_lagged) if mm_lagged is not None else None
        # --- S, recip, th ---
        Sth = [psl.tile([72, PSN], F32, tag="L", name=f"S{h}") for h in range(2)]
        for h in range(2):
            nc.tensor.matmul(Sth[h][64:72, 0:NC], cst["onesS"][:, h], E[0][:, 0:NC],
                             start=True, stop=False)
            nc.tensor.matmul(Sth[h][64:72, 0:NC], cst["onesS"][:, h], E[1][:, 0:NC],
                             start=False, stop=True)
        th = []
        for h in range(2):
            rS = work.tile([72, N], F32, name=f"rS{h}", tag=f"rS{h}")
            nc.vector.reciprocal(rS[64:72, 0:NC], Sth[h][64:72, 0:NC])
            t2 = work.tile([72, N], BF16, name=f"th{h}", tag=f"th{h}")
            with nc.allow_low_precision(reason="bf16 routing"):
                nc.vector.scalar_tensor_tensor(t2[64:72, 0:NC], tt[h][64:72, 0:NC],
                                               1.0, rS[64:72, 0:NC],
                                               op0=ALU.add, op1=ALU.mult)
            th.append(t2)
        # --- rtp, ar ---
        ar = []
        for s, half in enumerate("AB"):
            rtp = psl.tile([128, PSN], F32, tag="L", name=f"rtp{half}")
            nc.tensor.matmul(rtp[:, 0:NC], cst["rt"][64:72, 0], th[0][64:72, 0:NC],
                             start=True, stop=False)
            nc.tensor.matmul(rtp[:, 0:NC], cst["rt"][64:72, 1], th[1][64:72, 0:NC],
                             start=False, stop=True)
            arh = work.tile([128, N], BF16, name=f"ar{half}", tag=f"ar{half}", bufs=4)
            with nc.allow_low_precision(reason="bf16 routing"):
                nc.vector.tensor_mul(arh[:, 0:NC], E[s][:, 0:NC], rtp[:, 0:NC])
            ar.append(arh)
        # --- asum (+eps), r, bc, coeff ---
        asum = psl.tile([40, PSN], F32, tag="L", name="asum")
        nc.tensor.matmul(asum[0:8, 0:NC], cst["onesA"][:], ar[0][:, 0:NC],
                         start=True, stop=True, skip_group_check=True,
                         tile_position=(0, 0))
        nc.tensor.matmul(asum[32:40, 0:NC], cst["onesA"][:], ar[1][:, 0:NC],
                         start=True, stop=True, skip_group_check=True,
                         tile_position=(0, 32))
        as_eps = work.tile([40, N], BF16, tag="as_eps")
        with nc.allow_low_precision(reason="bf16 act channel"):
            nc.scalar.activation(as_eps[:, 0:NC], asum[0:40, 0:NC], AF.Copy,
                                 bias=EPS_ROUTE)
        r = r_t
        with nc.allow_low_precision(reason="bf16 routing"):
            nc.vector.reciprocal(r[0:8, 0:NC], as_eps[0:8, 0:NC])
            nc.vector.reciprocal(r[32:40, 0:NC], as_eps[32:40, 0:NC])
        coeff = []
        for s, half in enumerate("AB"):
            bc = psl.tile([128, PSN], F32, tag="L", name=f"bc{half}")
            nc.tensor.matmul(bc[:, 0:NC], cst["bsel"][32 * s:32 * s + 8, s],
                             r[32 * s:32 * s + 8, 0:NC], start=True, stop=True)
            ch = work.tile([128, N], BF16, name=f"co{half}", tag=f"co{half}", bufs=4)
            with nc.allow_low_precision(reason="bf16 routing"):
                nc.vector.tensor_mul(ch[:, 0:NC], ar[s][:, 0:NC], bc[:, 0:NC])
            coeff.append(ch)
        # --- per-o rep -> cp -> vote into channel-ordered packed psum ---
        pk_sb = []
        for h in range(2):
            packed = psl.tile([128, PSN], F32, tag="L", name=f"pk{h}")
            for o_loc in range(8):
                o = 8 * h + o_loc
                g = o_loc // 4
                rep = psr.tile([128, PSN], F32, tag="rep", name="rep")
                nc.tensor.matmul(rep[:, 0:NC], cst["sel2"][:, o_loc],
                                 coeff[h][:, 0:NC], start=True, stop=True)
                cp = work.tile([128, N], BF16, tag="cp")
                with nc.allow_low_precision(reason="bf16 routing"):
                    nc.vector.tensor_mul(cp[:, 0:NC], pose[:, 0:NC],
                                         rep[:, 0:NC])
                nc.tensor.matmul(packed[64 * g:64 * g + 36, 0:NC], cst["wp36"][:, o],
                                 cp[:, 0:NC], start=(o_loc % 4 == 0), stop=False,
                                 skip_group_check=True, tile_position=(0, 64 * g))
            nc.tensor.matmul(packed[0:100, 0:NC], cst["actsel"][32 * h:32 * h + 8, h],
                             as_eps[32 * h:32 * h + 8, 0:NC], start=False, stop=True,
                             skip_group_check=True, tile_position=(32 * h, 0))
            pk = work.tile([128, N], BF16, tag=f"pk{h}", bufs=3)
            with nc.allow_low_precision(reason="bf16 conv2 input"):
                nc.scalar.copy(pk[0:100, 0:NC], packed[0:100, 0:NC])
            pk_sb.append(pk)
        return pk_sb, c2ps_cur

    fin_prev = None
    for c in range(NCHUNK):
        mm_lagged = c - C2LAG if c >= C2LAG else None
        (pk_sb, c2ps_cur) = routing_chunk(c, fin_prev, mm_lagged)
        scatter_chunk(c, pk_sb)
        fin_prev = (mm_lagged, c2ps_cur) if c2ps_cur is not None else None
    # drain: remaining conv2 chunks
    if fin_prev is not None:
        conv2_fin(*fin_prev)
    for c in range(NCHUNK - C2LAG, NCHUNK):
        pss = conv2_mm(c)
        conv2_fin(c, pss)


def build_nc():
    nc = bacc.Bacc("TRN2", target_bir_lowering=False, debug=False)
    ins = {
        "x0": nc.dram_tensor("x0", [72, H, W], BF16, kind="ExternalInput").ap(),
        "x1": nc.dram_tensor("x1", [72, H, W], BF16, kind="ExternalInput").ap(),
    }
    for name, (shape, dt) in CONST_SPECS.items():
        ins[name] = nc.dram_tensor(name, shape, dt, kind="ExternalInput").ap()
    outs = {"out": nc.dram_tensor("out", [C_CAT, H, W], F32, kind="ExternalOutput").ap()}
    with tile.TileContext(nc) as tc:
        capsroute_kernel(tc, outs, ins)
    nc.compile()
    return nc


_NC_CACHE = {}


def _get_nc():
    if "nc" not in _NC_CACHE:
        _NC_CACHE["nc"] = build_nc()
    return _NC_CACHE["nc"]


def kernel(**inputs):
    """Full-batch entry point: shards batch 8 across 8 NeuronCores."""
    from concourse import bass_utils

    nc = _get_nc()
    consts = prep_constants(
        inputs["conv_route_w"].astype(np.float32),
        inputs["conv_route_gamma"].astype(np.float32),
        inputs["conv_route_beta"].astype(np.float32),
        inputs["W_pose"].astype(np.float32),
        inputs["W_gate"].astype(np.float32),
        inputs["b_gate"].astype(np.float32),
        inputs["spagg_w"].astype(np.float32),
        inputs["spagg_gamma"].astype(np.float32),
        inputs["spagg_beta"].astype(np.float32),
    )
    x0 = np.asarray(inputs["x0"]).astype(BF16_NP)
    x1 = np.asarray(inputs["x1"]).astype(BF16_NP)
    in_maps = []
    for b in range(8):
        m = dict(consts)
        m["x0"] = np.ascontiguousarray(x0[b])
        m["x1"] = np.ascontiguousarray(x1[b])
        in_maps.append(m)
    res = bass_utils.run_bass_kernel_spmd(nc, in_maps, core_ids=list(range(8)))
    out = np.stack([res.results[b]["out"] for b in range(8)], axis=0)
    return out.astype(np.float32)


# revision 7
# speedup vs baseline: 1.5436x; 1.0122x over previous
"""CapsRoute Trainium2 kernel v2: o-major routing, direct-to-channel votes.

Structure per row-chunk (nr rows, NC = nr*96 pixels):
  conv1 (18 mm) -> silu -> [o-major logits (4 mm) -> exp -> S (4 mm) ->
  recip -> th -> rtp (4 mm) -> ar -> asum (2 mm) -> as_eps -> r ->
  bc (2 mm) -> coeff] -> per-o [rep mm -> cp mul -> vote mm into
  channel-ordered packed psum] + act-insert mm -> Act copy -> 2 scatter
  DMAs per half into y2 -> conv2 (18 mm) at lag 3.
"""
import numpy as np
import concourse.bass as bass
import concourse.bacc as bacc
import concourse.tile as tile
from concourse import mybir
from concourse._compat import with_exitstack
from contextlib import ExitStack

K_CAT, P_CAT, K_OUT, P_OUT = 16, 8, 16, 8
C_CAT = 144
EPS_ROUTE = 1e-6
BN_EPS = 1e-5
H = W = 96
HP = WP = 98
ROWS_PER_CHUNK = 5
CHUNKS = [(5 * i, 5) for i in range(16)] + [(80 + 4 * i, 4) for i in range(4)]
NCHUNK = len(CHUNKS)
N = ROWS_PER_CHUNK * W  # 480
PSN = 512
C2LAG = 3

F32 = mybir.dt.float32
BF16 = mybir.dt.bfloat16
AF = mybir.ActivationFunctionType
ALU = mybir.AluOpType


def prep_constants(conv_route_w, conv_route_gamma, conv_route_beta,
                   W_pose, W_gate, b_gate, spagg_w, spagg_gamma, spagg_beta):
    out = {}
    scale1 = (conv_route_gamma / np.sqrt(1.0 + BN_EPS)).astype(np.float32)
    scale2 = (spagg_gamma / np.sqrt(1.0 + BN_EPS)).astype(np.float32)

    # conv1 lhsT [72, 2, 9, 72]: [cin_local, half, tap, col j]
    # col j: j<64 -> pose (k_loc=j//8, p=j%8); j>=64 -> act of k_loc=j-64
    c1 = np.zeros((72, 2, 9, 72), np.float32)
    w1 = conv_route_w * scale1[:, None, None, None]
    b1 = np.zeros((2, 72), np.float32)
    for h in range(2):
        for j in range(72):
            k_loc, ch = (j // 8, j % 8) if j < 64 else (j - 64, 8)
            cout = 72 * h + 9 * k_loc + ch
            for ci in range(9):
                for dy in range(3):
                    for dx in range(3):
                        c1[9 * k_loc + ci, h, 3 * dy + dx, j] = w1[cout, ci, dy, dx]
            b1[h, j] = conv_route_beta[cout]
    out["c1w"] = c1
    out["b1"] = b1

    # o-major logits lhsT [64, 4, 128]: slots (half-tile, rhs-half):
    # 0=(A,pose0) 1=(A,ptmp) 2=(B,pose0) 3=(B,ptmp); row r=8*k_loc+p,
    # col j=16*o_loc+k
    wg4 = np.zeros((64, 4, 128), np.float32)
    for r in range(64):
        kl, p = r // 8, r % 8
        for o_loc in range(8):
            wg4[r, 0, 16 * o_loc + kl] = W_gate[kl, o_loc, p]
            wg4[r, 1, 16 * o_loc + 8 + kl] = W_gate[8 + kl, o_loc, p]
            wg4[r, 2, 16 * o_loc + kl] = W_gate[kl, 8 + o_loc, p]
            wg4[r, 3, 16 * o_loc + 8 + kl] = W_gate[8 + kl, 8 + o_loc, p]
    out["wg4"] = wg4

    bg = np.asarray(b_gate).reshape(K_CAT, K_OUT)
    bgA = np.zeros((128, 1), np.float32)
    bgB = np.zeros((128, 1), np.float32)
    for o_loc in range(8):
        for k in range(16):
            bgA[16 * o_loc + k, 0] = bg[k, o_loc]
            bgB[16 * o_loc + k, 0] = bg[k, 8 + o_loc]
    out["bgA"] = bgA
    out["bgB"] = bgB

    # S: sum E over o per k-half: rows 16o+k -> col k-8h
    onesS = np.zeros((128, 2, 8), np.float32)
    for o_loc in range(8):
        for k in range(16):
            h = k // 8
            onesS[16 * o_loc + k, h, k - 8 * h] = 1.0
    out["onesS"] = onesS

    # rtp broadcast: rows 64+kl (th rows) -> cols 16o+ (8h+kl); 0.5 folds
    # the tanh->sigmoid affine
    rt = np.zeros((72, 2, 128), np.float32)
    for h in range(2):
        for kl in range(8):
            for o_loc in range(8):
                rt[64 + kl, h, 16 * o_loc + 8 * h + kl] = 0.5
    out["rt"] = rt

    # asum: rows 16o+k -> col o
    onesA = np.zeros((128, 8), np.float32)
    for o_loc in range(8):
        for k in range(16):
            onesA[16 * o_loc + k, o_loc] = 1.0
    out["onesA"] = onesA

    # late-normalize broadcast: r rows (A 0:8 / B 32:40, row 8 = ones) ->
    # packed channel cols; act cols get the constant-1 row
    rsel2 = np.zeros((72, 2, 100), np.float32)
    for h in range(2):
        for o_loc in range(8):
            for q in range(8):
                col = 64 * (o_loc // 4) + 9 * (o_loc % 4) + q
                rsel2[32 * h + o_loc, h, col] = 1.0
        for g in range(2):
            for j in range(4):
                rsel2[64, h, 64 * g + 9 * j + 8] = 1.0
    out["rsel2"] = rsel2

    # rep broadcast: rows 16o'+k -> cols 8k+p (all p) for o'==o_loc
    sel2 = np.zeros((128, 8, 128), np.float32)
    for o_loc in range(8):
        for k in range(16):
            for p in range(8):
                sel2[16 * o_loc + k, o_loc, 8 * k + p] = 1.0
    out["sel2"] = sel2

    # votes straight into channel order: rows 8k+p, per o: col
    # 9*(o_loc%4)+q in group o_loc//4
    wp36 = np.zeros((128, 16, 36), np.float32)
    for o in range(16):
        o_loc = o % 8
        for k in range(16):
            for p in range(8):
                for q in range(8):
                    wp36[8 * k + p, o, 9 * (o_loc % 4) + q] = W_pose[k, o, p, q]
    out["wp36"] = wp36

    # act insertion: rows of as_eps (A 0:8 / B 32:40) -> cols {9j+8, 64+9j+8}
    actsel = np.zeros((40, 2, 100), np.float32)
    for h in range(2):
        for o_loc in range(8):
            col = 64 * (o_loc // 4) + 9 * (o_loc % 4) + 8
            actsel[32 * h + o_loc, h, col] = 1.0
    out["actsel"] = actsel

    # conv2 lhsT [72, 2, 9, 72] natural channel order
    c2 = np.zeros((72, 2, 9, 72), np.float32)
    w2 = spagg_w * scale2[:, None, None, None]
    for h in range(2):
        for j in range(72):
            cout = 72 * h + j
            g_loc = j // 9
            for ci in range(9):
                for dy in range(3):
                    for dx in range(3):
                        c2[9 * g_loc + ci, h, 3 * dy + dx, j] = w2[cout, ci, dy, dx]
    out["c2w"] = c2
    out["b2"] = spagg_beta.reshape(2, 72).astype(np.float32)
    for name, (shape, dt) in CONST_SPECS.items():
        want = mybir.dt.np(dt)
        out[name] = np.ascontiguousarray(out[name]).astype(want)
    return out


CONST_SPECS = {
    "c1w": ([72, 2, 9, 72], BF16),
    "b1": ([2, 72], F32),
    "wg4": ([64, 4, 128], BF16),
    "bgA": ([128, 1], F32),
    "bgB": ([128, 1], F32),
    "onesS": ([128, 2, 8], BF16),
    "rt": ([72, 2, 128], BF16),
    "onesA": ([128, 8], BF16),
    "rsel2": ([72, 2, 100], BF16),
    "sel2": ([128, 8, 128], BF16),
    "wp36": ([128, 16, 36], BF16),
    "actsel": ([40, 2, 100], BF16),
    "c2w": ([72, 2, 9, 72], BF16),
    "b2": ([2, 72], F32),
}
BF16_NP = mybir.dt.np(BF16)
XROWS = [(0, 6), (6, 5), (11, 24), (35, 24), (59, 24), (83, 13)]


@with_exitstack
def capsroute_kernel(ctx: ExitStack, tc: tile.TileContext, outs, ins):
    nc = tc.nc
    out = outs["out"]

    singles = ctx.enter_context(tc.tile_pool(name="singles", bufs=1))
    xpool = ctx.enter_context(tc.tile_pool(name="xpool", bufs=1))
    y2pool = ctx.enter_context(tc.tile_pool(name="y2pool", bufs=1))
    work = ctx.enter_context(tc.tile_pool(name="work", bufs=5))
    psc = ctx.enter_context(tc.tile_pool(name="psc", bufs=2, space="PSUM"))
    psl = ctx.enter_context(tc.tile_pool(name="psl", bufs=2, space="PSUM"))
    psr = ctx.enter_context(tc.tile_pool(name="psr", bufs=2, space="PSUM"))
    pck = ctx.enter_context(tc.tile_pool(name="pck", bufs=2, space="PSUM"))

    cst = {}
    for name, (shape, dt) in CONST_SPECS.items():
        if name in ("b1", "b2"):
            continue
        t = singles.tile(shape, dt, name=f"{name}_c")
        cst[name] = t
    b1_t = [singles.tile([72, 1], F32, name=f"b1_{h}") for h in range(2)]
    b2_t = [singles.tile([72, 1], F32, name=f"b2_{h}") for h in range(2)]

    r_t = singles.tile([72, N], BF16, name="r_t")
    nc.vector.memset(r_t[:], 0.0)
    nc.vector.memset(r_t[64:72, :], 1.0)
    xpad = [xpool.tile([72, HP, WP], BF16, name=f"xpad{h}") for h in range(2)]
    y2 = [y2pool.tile([72, HP, WP], BF16, name=f"y2{h}") for h in range(2)]

    def pad_border(t):
        nc.vector.memset(t[:, 0, :], 0.0)
        nc.vector.memset(t[:, 97, :], 0.0)
        nc.vector.memset(t[:, :, 0:1], 0.0)
        nc.vector.memset(t[:, :, 97:98], 0.0)

    for h in range(2):
        pad_border(xpad[h])
        pad_border(y2[h])

    # load order tuned for startup: first x rows, conv1 consts, routing
    # consts, remaining x rows, conv2 consts
    nc.sync.dma_start(out=cst["c1w"][:, 0, 0:3], in_=ins["c1w"][:, 0, 0:3])
    nc.sync.dma_start(out=cst["c1w"][:, 0, 3:9], in_=ins["c1w"][:, 0, 3:9])
    r0, nr = XROWS[0]
    nc.gpsimd.dma_start(out=xpad[0][:, 1 + r0:1 + r0 + nr, 1:97],
                        in_=ins["x0"][:, r0:r0 + nr, :])
    nc.sync.dma_start(out=cst["c1w"][:, 1], in_=ins["c1w"][:, 1])
    nc.gpsimd.dma_start(out=xpad[1][:, 1 + r0:1 + r0 + nr, 1:97],
                        in_=ins["x1"][:, r0:r0 + nr, :])
    for h in range(2):
        nc.sync.dma_start(out=b1_t[h][:], in_=ins["b1"][h:h + 1, :].transpose([1, 0]))
    for name in ("wg4", "bgA", "bgB", "onesS", "rt", "onesA", "rsel2"):
        nc.sync.dma_start(out=cst[name][:], in_=ins[name][:])
    for h, xsrc in enumerate((ins["x0"], ins["x1"])):
        for r0, nr in XROWS[1:]:
            nc.gpsimd.dma_start(out=xpad[h][:, 1 + r0:1 + r0 + nr, 1:97],
                                in_=xsrc[:, r0:r0 + nr, :])
    for name in ("sel2", "wp36", "actsel"):
        nc.sync.dma_start(out=cst[name][:], in_=ins[name][:])
    nc.sync.dma_start(out=cst["c2w"][:], in_=ins["c2w"][:])
    for h in range(2):
        nc.sync.dma_start(out=b2_t[h][:], in_=ins["b2"][h:h + 1, :].transpose([1, 0]))

    def win(t, r0, nr, dy, dx):
        rs = 1 + r0 + dy
        return t[:, rs:rs + nr, 1 + dx:1 + dx + W]

    def conv2_mm(c):
        r0, nr = CHUNKS[c]
        NC = nr * W
        pss = []
        for h in range(2):
            ps = psc.tile([72, PSN], F32, tag="c2ps", name="c2ps")
            for tap in range(9):
                dy, dx = tap // 3 - 1, tap % 3 - 1
                nc.tensor.matmul(
                    ps[:, 0:NC], cst["c2w"][:, h, tap],
                    win(y2[h], r0, nr, dy, dx),
                    start=(tap == 0), stop=(tap == 8))
            pss.append(ps)
        return pss

    def conv2_fin(c, pss):
        r0, nr = CHUNKS[c]
        NC = nr * W
        for h in range(2):
            ob = work.tile([72, N], F32, tag="ob")
            nc.scalar.activation(ob[:, 0:NC], pss[h][:, 0:NC], AF.Silu, bias=b2_t[h][:])
            nc.sync.dma_start(
                out=out[72 * h:72 * h + 72, r0:r0 + nr, :],
                in_=ob[:, 0:NC].rearrange("p (r w) -> p r w", w=W))

    def scatter_chunk(c, pk_sb):
        r0, nr = CHUNKS[c]
        NC = nr * W
        for h in range(2):
            pk = pk_sb[h]
            for g in range(2):
                src = pk[64 * g:64 * g + 36, 0:NC].rearrange("p (r w) -> p r w", w=W)
                nc.sync.dma_start(
                    out=y2[h][36 * g:36 * g + 36, 1 + r0:1 + r0 + nr, 1:97],
                    in_=src)

    def routing_chunk(c, fin_prev, mm_lagged):
        r0, nr = CHUNKS[c]
        NC = nr * W
        # --- conv1 ---
        ps_h = []
        for h in range(2):
            ps = psc.tile([72, PSN], F32, tag="c1ps", name="c1ps")
            for tap in range(9):
                dy, dx = tap // 3 - 1, tap % 3 - 1
                nc.tensor.matmul(
                    ps[:, 0:NC], cst["c1w"][:, h, tap],
                    win(xpad[h], r0, nr, dy, dx),
                    start=(tap == 0), stop=(tap == 8))
            ps_h.append(ps)
        pose = work.tile([128, N], BF16, tag="pose", bufs=4)
        ptmp = work.tile([64, N], BF16, tag="ptmp")
        acty = [work.tile([72, N], F32, name=f"acty{h}", tag=f"acty{h}") for h in range(2)]
        nc.scalar.activation(pose[0:64, 0:NC], ps_h[0][0:64, 0:NC], AF.Silu,
                             bias=b1_t[0][0:64, :])
        nc.scalar.activation(ptmp[:, 0:NC], ps_h[1][0:64, 0:NC], AF.Silu,
                             bias=b1_t[1][0:64, :])
        for h in range(2):
            nc.scalar.activation(acty[h][64:72, 0:NC], ps_h[h][64:72, 0:NC],
                                 AF.Silu, bias=b1_t[h][64:72, :])
        nc.sync.dma_start(out=pose[64:128, 0:NC], in_=ptmp[:, 0:NC])
        if fin_prev is not None:
            conv2_fin(*fin_prev)
        # tanh(x/2) = 2*sigmoid(x)-1; shares the exp act-table set
        tt = [work.tile([72, N], BF16, name=f"t{h}", tag=f"t{h}") for h in range(2)]
        for h in range(2):
            with nc.allow_low_precision(reason="bf16 routing"):
                nc.scalar.activation(tt[h][64:72, 0:NC], acty[h][64:72, 0:NC],
                                     AF.Tanh, scale=0.5)
        # --- o-major logits + exp ---
        E = []
        for s, half in enumerate("AB"):
            L = psl.tile([128, PSN], F32, tag="L", name=f"L{half}")
            nc.tensor.matmul(L[:, 0:NC], cst["wg4"][:, 2 * s], pose[0:64, 0:NC],
                             start=True, stop=False)
            nc.tensor.matmul(L[:, 0:NC], cst["wg4"][:, 2 * s + 1], ptmp[:, 0:NC],
                             start=False, stop=True)
            Eh = work.tile([128, N], BF16, tag=f"E{half}", bufs=4)
            with nc.allow_low_precision(reason="bf16 routing"):
                nc.scalar.activation(Eh[:, 0:NC], L[:, 0:NC], AF.Exp,
                                     bias=cst["bg" + half][:])
            E.append(Eh)
        c2ps_cur = conv2_mm(mm_lagged) if mm_lagged is not None else None
        # --- S, recip, th ---
        Sth = [psl.tile([72, PSN], F32, tag="L", name=f"S{h}") for h in range(2)]
        for h in range(2):
            nc.tensor.matmul(Sth[h][64:72, 0:NC], cst["onesS"][:, h], E[0][:, 0:NC],
                             start=True, stop=False)
            nc.tensor.matmul(Sth[h][64:72, 0:NC], cst["onesS"][:, h], E[1][:, 0:NC],
                             start=False, stop=True)
        th = []
        for h in range(2):
            rS = work.tile([72, N], F32, name=f"rS{h}", tag=f"rS{h}")
            nc.vector.reciprocal(rS[64:72, 0:NC], Sth[h][64:72, 0:NC])
            t2 = work.tile([72, N], BF16, name=f"th{h}", tag=f"th{h}")
            with nc.allow_low_precision(reason="bf16 routing"):
                nc.vector.scalar_tensor_tensor(t2[64:72, 0:NC], tt[h][64:72, 0:NC],
                                               1.0, rS[64:72, 0:NC],
                                               op0=ALU.add, op1=ALU.mult)
            th.append(t2)
        # --- rtp, ar ---
        ar = []
        for s, half in enumerate("AB"):
            rtp = psl.tile([128, PSN], F32, tag="L", name=f"rtp{half}")
            nc.tensor.matmul(rtp[:, 0:NC], cst["rt"][64:72, 0], th[0][64:72, 0:NC],
                             start=True, stop=False)
            nc.tensor.matmul(rtp[:, 0:NC], cst["rt"][64:72, 1], th[1][64:72, 0:NC],
                             start=False, stop=True)
            arh = work.tile([128, N], BF16, name=f"ar{half}", tag=f"ar{half}", bufs=4)
            with nc.allow_low_precision(reason="bf16 routing"):
                nc.vector.tensor_mul(arh[:, 0:NC], E[s][:, 0:NC], rtp[:, 0:NC])
            ar.append(arh)
        # --- asum (+eps), r, bc, coeff ---
        asum = psl.tile([40, PSN], F32, tag="L", name="asum")
        nc.tensor.matmul(asum[0:8, 0:NC], cst["onesA"][:], ar[0][:, 0:NC],
                         start=True, stop=True, skip_group_check=True,
                         tile_position=(0, 0))
        nc.tensor.matmul(asum[32:40, 0:NC], cst["onesA"][:], ar[1][:, 0:NC],
                         start=True, stop=True, skip_group_check=True,
                         tile_position=(0, 32))
        as_eps = work.tile([40, N], BF16, tag="as_eps")
        with nc.allow_low_precision(reason="bf16 act channel"):
            nc.scalar.activation(as_eps[:, 0:NC], asum[0:40, 0:NC], AF.Copy,
                                 bias=EPS_ROUTE)
        r = r_t
        with nc.allow_low_precision(reason="bf16 routing"):
            nc.vector.reciprocal(r[0:8, 0:NC], as_eps[0:8, 0:NC])
            nc.vector.reciprocal(r[32:40, 0:NC], as_eps[32:40, 0:NC])
        coeff = []
        for s, half in enumerate("AB"):
            bc = psl.tile([128, PSN], F32, tag="L", name=f"bc{half}")
            nc.tensor.matmul(bc[:, 0:NC], cst["bsel"][32 * s:32 * s + 8, s],
                             r[32 * s:32 * s + 8, 0:NC], start=True, stop=True)
            ch = work.tile([128, N], BF16, name=f"co{half}", tag=f"co{half}", bufs=4)
            with nc.allow_low_precision(reason="bf16 routing"):
                nc.vector.tensor_mul(ch[:, 0:NC], ar[s][:, 0:NC], bc[:, 0:NC])
            coeff.append(ch)
        # --- per-o rep -> cp -> vote into channel-ordered packed psum ---
        pk_sb = []
        for h in range(2):
            packed = psl.tile([128, PSN], F32, tag="L", name=f"pk{h}")
            for o_loc in range(8):
                o = 8 * h + o_loc
                g = o_loc // 4
                rep = psr.tile([128, PSN], F32, tag="rep", name="rep")
                nc.tensor.matmul(rep[:, 0:NC], cst["sel2"][:, o_loc],
                                 coeff[h][:, 0:NC], start=True, stop=True)
                cp = work.tile([128, N], BF16, tag="cp", bufs=6)
                with nc.allow_low_precision(reason="bf16 routing"):
                    nc.vector.tensor_mul(cp[:, 0:NC], pose[:, 0:NC],
                                         rep[:, 0:NC])
                nc.tensor.matmul(packed[64 * g:64 * g + 36, 0:NC], cst["wp36"][:, o],
                                 cp[:, 0:NC], start=(o_loc % 4 == 0), stop=False,
                                 skip_group_check=True, tile_position=(0, 64 * g))
            nc.tensor.matmul(packed[0:100, 0:NC], cst["actsel"][32 * h:32 * h + 8, h],
                             as_eps[32 * h:32 * h + 8, 0:NC], start=False, stop=True,
                             skip_group_check=True, tile_position=(32 * h, 0))
            pk = work.tile([128, N], BF16, tag=f"pk{h}", bufs=3)
            with nc.allow_low_precision(reason="bf16 conv2 input"):
                nc.scalar.copy(pk[0:100, 0:NC], packed[0:100, 0:NC])
            pk_sb.append(pk)
        return pk_sb, c2ps_cur

    fin_prev = None
    for c in range(NCHUNK):
        mm_lagged = c - C2LAG if c >= C2LAG else None
        (pk_sb, c2ps_cur) = routing_chunk(c, fin_prev, mm_lagged)
        scatter_chunk(c, pk_sb)
        fin_prev = (mm_lagged, c2ps_cur) if c2ps_cur is not None else None
    # drain: remaining conv2 chunks
    if fin_prev is not None:
        conv2_fin(*fin_prev)
    for c in range(NCHUNK - C2LAG, NCHUNK):
        pss = conv2_mm(c)
        conv2_fin(c, pss)


def build_nc():
    nc = bacc.Bacc("TRN2", target_bir_lowering=False, debug=False)
    ins = {
        "x0": nc.dram_tensor("x0", [72, H, W], BF16, kind="ExternalInput").ap(),
        "x1": nc.dram_tensor("x1", [72, H, W], BF16, kind="ExternalInput").ap(),
    }
    for name, (shape, dt) in CONST_SPECS.items():
        ins[name] = nc.dram_tensor(name, shape, dt, kind="ExternalInput").ap()
    outs = {"out": nc.dram_tensor("out", [C_CAT, H, W], F32, kind="ExternalOutput").ap()}
    with tile.TileContext(nc) as tc:
        capsroute_kernel(tc, outs, ins)
    nc.compile()
    return nc


_NC_CACHE = {}


def _get_nc():
    if "nc" not in _NC_CACHE:
        _NC_CACHE["nc"] = build_nc()
    return _NC_CACHE["nc"]


def kernel(**inputs):
    """Full-batch entry point: shards batch 8 across 8 NeuronCores."""
    from concourse import bass_utils

    nc = _get_nc()
    consts = prep_constants(
        inputs["conv_route_w"].astype(np.float32),
        inputs["conv_route_gamma"].astype(np.float32),
        inputs["conv_route_beta"].astype(np.float32),
        inputs["W_pose"].astype(np.float32),
        inputs["W_gate"].astype(np.float32),
        inputs["b_gate"].astype(np.float32),
        inputs["spagg_w"].astype(np.float32),
        inputs["spagg_gamma"].astype(np.float32),
        inputs["spagg_beta"].astype(np.float32),
    )
    x0 = np.asarray(inputs["x0"]).astype(BF16_NP)
    x1 = np.asarray(inputs["x1"]).astype(BF16_NP)
    in_maps = []
    for b in range(8):
        m = dict(consts)
        m["x0"] = np.ascontiguousarray(x0[b])
        m["x1"] = np.ascontiguousarray(x1[b])
        in_maps.append(m)
    res = bass_utils.run_bass_kernel_spmd(nc, in_maps, core_ids=list(range(8)))
    out = np.stack([res.results[b]["out"] for b in range(8)], axis=0)
    return out.astype(np.float32)
